# revision 2
# baseline (speedup 1.0000x reference)
"""Trainium2 Bass kernel for nn_ConditionalDLFactorized16 (moe_routing).

Data-parallel over tokens: the T*B=16384 tokens are split into 8 contiguous
slabs of 2048, one per NeuronCore. Weights (pw_w/centroids/map_w) are
replicated. The only cross-core quantity is the scalar loss, combined on host
from per-core partial sums.

Per-core math (tokens n local):
  k[n,r]   = x[n,:] @ map_w.T + map_b                    (fp32r matmuls)
  g[n,e]   = k[n,:] @ centroids.T - 0.5*||c_e||^2 + log prior_e
             (bias row folded into an augmented [65,8] centroid matrix,
              with k augmented by a constant-1 row)
  resp     = softmax_e(g)     (the -0.5*kk and const terms cancel in softmax)
  y[n,:]   = sum_e resp[n,e] * (x[n,:] @ W'_e.T) + bias_b
             where W'_e = pw_w[e] + bias_w  (valid because sum_e resp = 1)
  loss parts: S1 = sum_n ||k_n||^2,  S2 = sum_n log(sum_e exp(g[n,e]))
  loss     = 0.1 * (0.5*S1 + N*(RD/2)*log(2*pi) - S2)   (TAU=1)

Expert matmuls run as fp32r (full PE rate at free-dim 512, ~2e-4 rel err)
with tokens on PSUM partitions so the resp-mixing is per-partition-scalar
work on the Scalar/Vector engines.
"""

import math
import sys

import numpy as np

for _p in ("/opt/trn_rl_repo", "/opt/pypackages"):
    if _p not in sys.path:
        sys.path.append(_p)

import concourse.bacc as bacc
import concourse.mybir as mybir
import concourse.tile as tile
from concourse.bass_utils import run_bass_kernel_spmd

F32 = mybir.dt.float32
F32R = mybir.dt.float32r
AF = mybir.ActivationFunctionType

T, B, C, O, RD, NE = 2048, 8, 512, 512, 64, 8
TAU, COMMIT = 1.0, 0.1
NCORES = 8
NLOC = T * B // NCORES          # 2048 tokens per core
NT = NLOC // 128                # 16 token tiles of 128
NST = NLOC // 512               # 4 streaming tiles of 512
KC = C // 128                   # 4 contraction chunks

_nc_cache = None


def _build():
    nc = bacc.Bacc(trn_type="TRN2", target_bir_lowering=False)

    xt = nc.dram_tensor("xt", [128, KC, NLOC], F32R, kind="ExternalInput")
    wt = nc.dram_tensor("wt", [128, NE, KC, O], F32R, kind="ExternalInput")
    mapw = nc.dram_tensor("mapw", [128, KC, RD], F32R, kind="ExternalInput")
    mapb = nc.dram_tensor("mapb", [RD, 1], F32, kind="ExternalInput")
    cent = nc.dram_tensor("cent", [RD + 1, NE], F32, kind="ExternalInput")
    biasb = nc.dram_tensor("biasb", [O], F32, kind="ExternalInput")

    y = nc.dram_tensor("y", [NLOC, O], F32, kind="ExternalOutput")
    lparts = nc.dram_tensor("lparts", [1, 2], F32, kind="ExternalOutput")

    with tile.TileContext(nc) as tc:
        with tc.tile_pool(name="big", bufs=1) as big, \
             tc.tile_pool(name="small", bufs=1) as small, \
             tc.tile_pool(name="acc_p", bufs=3) as acc_p, \
             tc.tile_pool(name="tmp_p", bufs=4) as tmp_p, \
             tc.tile_pool(name="route_p", bufs=16) as route_p, \
             tc.tile_pool(name="ps_z", bufs=4, space="PSUM") as ps_z, \
             tc.tile_pool(name="ps_kt", bufs=2, space="PSUM") as ps_kt, \
             tc.tile_pool(name="ps_s", bufs=2, space="PSUM") as ps_s:

            # ---- resident inputs ----
            xt_sb = big.tile([128, KC, NLOC], F32R)
            wt_sb = big.tile([128, NE, KC, O], F32R)
            mapw_sb = small.tile([128, KC, RD], F32R)
            mapb_sb = small.tile([RD, 1], F32)
            cent_sb = small.tile([RD + 1, NE], F32)
            biasbc = small.tile([128, O], F32)
            nc.sync.dma_start(xt_sb[:], xt[:])
            nc.sync.dma_start(wt_sb[:], wt[:])
            nc.sync.dma_start(mapw_sb[:], mapw[:])
            nc.sync.dma_start(mapb_sb[:], mapb[:])
            nc.sync.dma_start(cent_sb[:], cent[:])
            nc.sync.dma_start(biasbc[:], biasb[:].partition_broadcast(128))

            ones64 = small.tile([RD, 1], F32)
            ones128 = small.tile([128, 1], F32)
            nc.vector.memset(ones64[:], 1.0)
            nc.vector.memset(ones128[:], 1.0)

            # k^T in [r, tok] layout, augmented with a constant-1 row 64
            ktsb = small.tile([RD + 1, NLOC], F32)
            nc.vector.memset(ktsb[RD : RD + 1, :], 1.0)
            sq_cols = small.tile([RD, NST], F32)
            lse_cols = small.tile([128, NT], F32)

            # ---- routing: k = x @ map_w.T + map_b (tokens on free dim) ----
            for st in range(NST):
                kt_ps = ps_kt.tile([RD, 512], F32)
                for kc in range(KC):
                    nc.tensor.matmul(
                        kt_ps[:],
                        mapw_sb[:, kc, :],
                        xt_sb[:, kc, st * 512 : (st + 1) * 512],
                        start=(kc == 0),
                        stop=(kc == KC - 1),
                    )
                # k rows 0..63 with map_b added (per-partition scalar)
                nc.vector.tensor_scalar_add(
                    ktsb[:RD, st * 512 : (st + 1) * 512], kt_ps[:], mapb_sb[:]
                )
                # sum_tok (k + map_b)^2 per r-partition, for the loss
                ksq = tmp_p.tile([RD, 512], F32, tag="ksq")
                nc.scalar.activation(
                    ksq[:], kt_ps[:], AF.Square, bias=mapb_sb[:], scale=1.0,
                    accum_out=sq_cols[:, st : st + 1],
                )

            # ---- routing: g, expg, resp per 128-token tile ----
            resp_tiles = []
            for t in range(NT):
                g_ps = ps_s.tile([128, NE], F32, tag="g")
                nc.tensor.matmul(
                    g_ps[:],
                    ktsb[:, t * 128 : (t + 1) * 128],
                    cent_sb[:],
                    start=True,
                    stop=True,
                )
                expg = route_p.tile([128, NE], F32, tag="expg")
                sumexp = route_p.tile([128, 1], F32, tag="sumexp")
                nc.scalar.activation(
                    expg[:], g_ps[:], AF.Exp, bias=0.0, scale=1.0,
                    accum_out=sumexp[:],
                )
                inv = route_p.tile([128, 1], F32, tag="inv")
                nc.vector.reciprocal(inv[:], sumexp[:])
                nc.scalar.activation(
                    lse_cols[:, t : t + 1], sumexp[:], AF.Ln, bias=0.0, scale=1.0
                )
                resp = route_p.tile([128, NE], F32, tag="resp")
                nc.vector.tensor_scalar_mul(resp[:], expg[:], inv[:])
                resp_tiles.append(resp)

            # ---- expert matmuls + resp mixing ----
            for t in range(NT):
                resp = resp_tiles[t]
                acc = acc_p.tile([128, O], F32, tag="acc")
                for e in range(NE):
                    z_ps = ps_z.tile([128, O], F32, tag="z")
                    for kc in range(KC):
                        nc.tensor.matmul(
                            z_ps[:],
                            xt_sb[:, kc, t * 128 : (t + 1) * 128],
                            wt_sb[:, e, kc, :],
                            start=(kc == 0),
                            stop=(kc == KC - 1),
                        )
                    tmp = tmp_p.tile([128, O], F32, tag="zmix")
                    nc.scalar.activation(
                        tmp[:], z_ps[:], AF.Copy, bias=0.0, scale=resp[:, e : e + 1]
                    )
                    if e == 0:
                        nc.vector.tensor_add(acc[:], tmp[:], biasbc[:])
                    else:
                        nc.vector.tensor_add(acc[:], acc[:], tmp[:])
                nc.sync.dma_start(y[t * 128 : (t + 1) * 128, :], acc[:])

            # ---- loss partial sums ----
            sq_red = small.tile([RD, 1], F32)
            lse_red = small.tile([128, 1], F32)
            nc.vector.reduce_sum(sq_red[:], sq_cols[:], axis=mybir.AxisListType.X)
            nc.vector.reduce_sum(lse_red[:], lse_cols[:], axis=mybir.AxisListType.X)
            l1_ps = ps_s.tile([1, 1], F32, tag="g")
            l2_ps = ps_s.tile([1, 1], F32, tag="g")
            nc.tensor.matmul(l1_ps[:], sq_red[:], ones64[:], start=True, stop=True)
            nc.tensor.matmul(l2_ps[:], lse_red[:], ones128[:], start=True, stop=True)
            lp_sb = small.tile([1, 2], F32)
            nc.vector.tensor_copy(lp_sb[:, 0:1], l1_ps[:])
            nc.vector.tensor_copy(lp_sb[:, 1:2], l2_ps[:])
            nc.sync.dma_start(lparts[:], lp_sb[:])

    nc.finalize()
    return nc


def _get_nc():
    global _nc_cache
    if _nc_cache is None:
        _nc_cache = _build()
    return _nc_cache


def _prep_inputs(x, key_feat, map_w, map_b, centroids, prior, pw_w, bias_w, bias_b):
    """Host-side layout: shard tokens, transpose into c-on-partition chunks."""
    xf = np.ascontiguousarray(x, dtype=np.float32).reshape(T * B, C)

    Wp = pw_w.reshape(NE, O, C).astype(np.float32) + bias_w.astype(np.float32)[None]
    # wt[ci, e, kc, o] = Wp[e, o, kc*128+ci]
    wt = np.ascontiguousarray(Wp.reshape(NE, O, KC, 128).transpose(3, 0, 2, 1))
    # mapw[ci, kc, r] = map_w[r, kc*128+ci]
    mapw = np.ascontiguousarray(
        map_w.astype(np.float32).reshape(RD, KC, 128).transpose(2, 1, 0)
    )
    mapb = np.ascontiguousarray(map_b.astype(np.float32).reshape(RD, 1))
    cent = np.concatenate(
        [
            centroids.astype(np.float32).T,
            (-0.5 * (centroids.astype(np.float32) ** 2).sum(1)
             + np.log(prior[0].astype(np.float32)))[None, :],
        ],
        axis=0,
    ).astype(np.float32)
    biasb = np.ascontiguousarray(bias_b, dtype=np.float32)

    in_maps = []
    for c in range(NCORES):
        xc = xf[c * NLOC : (c + 1) * NLOC]
        # xt[ci, kc, n] = xc[n, kc*128+ci]
        xtc = np.ascontiguousarray(xc.reshape(NLOC, KC, 128).transpose(2, 1, 0))
        in_maps.append(
            dict(xt=xtc, wt=wt, mapw=mapw, mapb=mapb, cent=cent, biasb=biasb)
        )
    return in_maps


def _run(inputs, trace=False):
    in_maps = _prep_inputs(**inputs)
    nc = _get_nc()
    res = run_bass_kernel_spmd(
        nc, in_maps, core_ids=list(range(NCORES)), trace=trace
    )
    y = np.concatenate([res.results[c]["y"] for c in range(NCORES)], axis=0)
    y = y.reshape(T, B, O)
    S1 = sum(float(res.results[c]["lparts"][0, 0]) for c in range(NCORES))
    S2 = sum(float(res.results[c]["lparts"][0, 1]) for c in range(NCORES))
    loss = np.float32(
        COMMIT * (0.5 * TAU * S1
                  + T * B * (RD / 2.0) * math.log(2.0 * math.pi * TAU)
                  - S2)
    )
    return (y, loss), res


def kernel(**inputs):
    outputs, _ = _run(inputs, trace=False)
    return outputs


# revision 3
# speedup vs baseline: 1.3339x; 1.3339x over previous
"""Trainium2 Bass kernel for nn_ConditionalDLFactorized16 (moe_routing).

Data-parallel over tokens: the T*B=16384 tokens are split into 8 contiguous
slabs of 2048, one per NeuronCore. Weights (pw_w/centroids/map_w) are
replicated. The only cross-core quantity is the scalar loss, combined on host
from per-core partial sums.

Per-core math (tokens n local):
  k[n,r]   = x[n,:] @ map_w.T + map_b                    (fp32r matmuls)
  g[n,e]   = k[n,:] @ centroids.T - 0.5*||c_e||^2 + log prior_e
             (bias row folded into an augmented [65,8] centroid matrix,
              with k augmented by a constant-1 row)
  resp     = softmax_e(g)     (the -0.5*kk and const terms cancel in softmax)
  y[n,:]   = sum_e resp[n,e] * (x[n,:] @ W'_e.T) + bias_b
             where W'_e = pw_w[e] + bias_w  (valid because sum_e resp = 1)
  loss parts: S1 = sum_n ||k_n||^2,  S2 = sum_n log(sum_e exp(g[n,e]))
  loss     = 0.1 * (0.5*S1 + N*(RD/2)*log(2*pi) - S2)   (TAU=1)

Perf notes:
  - Expert matmuls in fp32r: full PE rate at free-dim 512, ~2e-4 rel err.
  - Tokens sit on PSUM partitions so the whole resp-mix per expert is ONE
    DVE scalar_tensor_tensor: acc = (z_psum * resp_col) + acc.
  - Inputs stream in small chunks (weights per expert) so PE starts ~4us in
    instead of waiting ~35us for the full 12.4MB load.
  - ACT does only 16 Exp (+accumulated sumexp) and one [128,16] Ln at the
    end: activation-table loads stay at 2 instead of 28.
"""

import math
import sys

import numpy as np

for _p in ("/opt/trn_rl_repo", "/opt/pypackages"):
    if _p not in sys.path:
        sys.path.append(_p)

import concourse.bacc as bacc
import concourse.mybir as mybir
import concourse.tile as tile
from concourse.bass_utils import run_bass_kernel_spmd

F32 = mybir.dt.float32
F32R = mybir.dt.float32r
AF = mybir.ActivationFunctionType
ALU = mybir.AluOpType

T, B, C, O, RD, NE = 2048, 8, 512, 512, 64, 8
TAU, COMMIT = 1.0, 0.1
NCORES = 8
NLOC = T * B // NCORES          # 2048 tokens per core
NT = NLOC // 128                # 16 token tiles of 128
NST = NLOC // 512               # 4 streaming tiles of 512
KC = C // 128                   # 4 contraction chunks

_nc_cache = None


def _build():
    nc = bacc.Bacc(trn_type="TRN2", target_bir_lowering=False)

    xt = nc.dram_tensor("xt", [128, NST, KC, 512], F32R, kind="ExternalInput")
    wt = nc.dram_tensor("wt", [128, NE, KC, O], F32R, kind="ExternalInput")
    mapw = nc.dram_tensor("mapw", [128, KC, RD], F32R, kind="ExternalInput")
    mapb = nc.dram_tensor("mapb", [RD, 1], F32, kind="ExternalInput")
    cent = nc.dram_tensor("cent", [RD + 1, NE], F32, kind="ExternalInput")
    biasb = nc.dram_tensor("biasb", [O], F32, kind="ExternalInput")

    y = nc.dram_tensor("y", [NLOC, O], F32, kind="ExternalOutput")
    lparts = nc.dram_tensor("lparts", [1, 2], F32, kind="ExternalOutput")

    with tile.TileContext(nc) as tc:
        with tc.tile_pool(name="big", bufs=1) as big, \
             tc.tile_pool(name="small", bufs=1) as small, \
             tc.tile_pool(name="acc_p", bufs=3) as acc_p, \
             tc.tile_pool(name="route_p", bufs=16) as route_p, \
             tc.tile_pool(name="ps_z", bufs=6, space="PSUM") as ps_z, \
             tc.tile_pool(name="ps_s", bufs=1, space="PSUM") as ps_s:

            # ---- resident inputs; small tensors first, then streamed chunks
            mapw_sb = small.tile([128, KC, RD], F32R)
            mapb_sb = small.tile([RD, 1], F32)
            cent_sb = small.tile([RD + 1, NE], F32)
            biasbc = small.tile([128, O], F32)
            nc.sync.dma_start(mapw_sb[:], mapw[:])
            nc.sync.dma_start(mapb_sb[:], mapb[:])
            nc.sync.dma_start(cent_sb[:], cent[:])
            nc.sync.dma_start(biasbc[:], biasb[:].partition_broadcast(128))

            xt_sb = big.tile([128, NST, KC, 512], F32R)
            wt_sb = big.tile([128, NE, KC, O], F32R)
            for st in range(NST):
                nc.sync.dma_start(xt_sb[:, st], xt[:, st])
            for e in range(NE):
                nc.sync.dma_start(wt_sb[:, e], wt[:, e])

            ones64 = small.tile([RD, 1], F32)
            ones128 = small.tile([128, 1], F32)
            nc.vector.memset(ones64[:], 1.0)
            nc.vector.memset(ones128[:], 1.0)

            # k^T in [r, tok] layout, augmented with a constant-1 row 64
            ktsb = small.tile([RD + 1, NLOC], F32)
            nc.vector.memset(ktsb[RD : RD + 1, :], 1.0)
            sq_cols = small.tile([RD, NST], F32)
            sumexp_cols = small.tile([128, NT], F32)

            # ---- routing: k = x @ map_w.T + map_b (tokens on free dim) ----
            for st in range(NST):
                kt_ps = ps_s.tile([RD, 512], F32, tag="kt")
                for kc in range(KC):
                    nc.tensor.matmul(
                        kt_ps[:],
                        mapw_sb[:, kc, :],
                        xt_sb[:, st, kc, :],
                        start=(kc == 0),
                        stop=(kc == KC - 1),
                    )
                ksl = ktsb[:RD, st * 512 : (st + 1) * 512]
                nc.vector.tensor_scalar_add(ksl, kt_ps[:], mapb_sb[:])
                # sum_tok k^2 per r-partition (loss S1), square fused with sum
                ksq = route_p.tile([RD, 512], F32, tag="ksq")
                nc.vector.scalar_tensor_tensor(
                    ksq[:], ksl, 1.0, ksl, op0=ALU.mult, op1=ALU.mult,
                    accum_out=sq_cols[:, st : st + 1],
                )

            # ---- routing: g, expg, resp per 128-token tile ----
            resp_tiles = []
            for t in range(NT):
                g_ps = ps_s.tile([128, NE], F32, tag="g")
                nc.tensor.matmul(
                    g_ps[:],
                    ktsb[:, t * 128 : (t + 1) * 128],
                    cent_sb[:],
                    start=True,
                    stop=True,
                )
                expg = route_p.tile([128, NE], F32, tag="expg")
                nc.scalar.activation(
                    expg[:], g_ps[:], AF.Exp, bias=0.0, scale=1.0,
                    accum_out=sumexp_cols[:, t : t + 1],
                )
                inv = route_p.tile([128, 1], F32, tag="inv")
                nc.vector.reciprocal(inv[:], sumexp_cols[:, t : t + 1])
                resp = route_p.tile([128, NE], F32, tag="resp")
                nc.vector.tensor_scalar_mul(resp[:], expg[:], inv[:])
                resp_tiles.append(resp)

            # ---- expert matmuls + resp mixing (one DVE op per expert) ----
            for t in range(NT):
                resp = resp_tiles[t]
                st, m0 = t // (NT // NST), (t % (NT // NST)) * 128
                acc = acc_p.tile([128, O], F32, tag="acc")
                for e in range(NE):
                    z_ps = ps_z.tile([128, O], F32, tag="z")
                    for kc in range(KC):
                        nc.tensor.matmul(
                            z_ps[:],
                            xt_sb[:, st, kc, m0 : m0 + 128],
                            wt_sb[:, e, kc, :],
                            start=(kc == 0),
                            stop=(kc == KC - 1),
                        )
                    nc.vector.scalar_tensor_tensor(
                        acc[:], z_ps[:], resp[:, e : e + 1],
                        biasbc[:] if e == 0 else acc[:],
                        op0=ALU.mult, op1=ALU.add,
                    )
                nc.sync.dma_start(y[t * 128 : (t + 1) * 128, :], acc[:])

            # ---- loss partial sums (single Ln at the very end) ----
            lse16 = small.tile([128, NT], F32)
            nc.scalar.activation(lse16[:], sumexp_cols[:], AF.Ln, bias=0.0, scale=1.0)
            sq_red = small.tile([RD, 1], F32)
            lse_red = small.tile([128, 1], F32)
            nc.vector.reduce_sum(sq_red[:], sq_cols[:], axis=mybir.AxisListType.X)
            nc.vector.reduce_sum(lse_red[:], lse16[:], axis=mybir.AxisListType.X)
            l1_ps = ps_s.tile([1, 1], F32, tag="kt")
            l2_ps = ps_s.tile([1, 1], F32, tag="g")
            nc.tensor.matmul(l1_ps[:], sq_red[:], ones64[:], start=True, stop=True)
            nc.tensor.matmul(l2_ps[:], lse_red[:], ones128[:], start=True, stop=True)
            lp_sb = small.tile([1, 2], F32)
            nc.vector.tensor_copy(lp_sb[:, 0:1], l1_ps[:])
            nc.vector.tensor_copy(lp_sb[:, 1:2], l2_ps[:])
            nc.sync.dma_start(lparts[:], lp_sb[:])

    nc.finalize()
    return nc


def _get_nc():
    global _nc_cache
    if _nc_cache is None:
        _nc_cache = _build()
    return _nc_cache


def _prep_inputs(x, key_feat, map_w, map_b, centroids, prior, pw_w, bias_w, bias_b):
    """Host-side layout: shard tokens, transpose into c-on-partition chunks."""
    xf = np.ascontiguousarray(x, dtype=np.float32).reshape(T * B, C)

    Wp = pw_w.reshape(NE, O, C).astype(np.float32) + bias_w.astype(np.float32)[None]
    # wt[ci, e, kc, o] = Wp[e, o, kc*128+ci]
    wt = np.ascontiguousarray(Wp.reshape(NE, O, KC, 128).transpose(3, 0, 2, 1))
    # mapw[ci, kc, r] = map_w[r, kc*128+ci]
    mapw = np.ascontiguousarray(
        map_w.astype(np.float32).reshape(RD, KC, 128).transpose(2, 1, 0)
    )
    mapb = np.ascontiguousarray(map_b.astype(np.float32).reshape(RD, 1))
    cent = np.concatenate(
        [
            centroids.astype(np.float32).T,
            (-0.5 * (centroids.astype(np.float32) ** 2).sum(1)
             + np.log(prior[0].astype(np.float32)))[None, :],
        ],
        axis=0,
    ).astype(np.float32)
    biasb = np.ascontiguousarray(bias_b, dtype=np.float32)

    in_maps = []
    for c in range(NCORES):
        xc = xf[c * NLOC : (c + 1) * NLOC]
        # xt[ci, st, kc, m] = xc[st*512 + m, kc*128+ci]
        xtc = np.ascontiguousarray(
            xc.reshape(NST, 512, KC, 128).transpose(3, 0, 2, 1)
        )
        in_maps.append(
            dict(xt=xtc, wt=wt, mapw=mapw, mapb=mapb, cent=cent, biasb=biasb)
        )
    return in_maps


def _run(inputs, trace=False, tmpdir=None):
    in_maps = _prep_inputs(**inputs)
    nc = _get_nc()
    res = run_bass_kernel_spmd(
        nc, in_maps, core_ids=list(range(NCORES)), trace=trace, tmpdir=tmpdir
    )
    y = np.concatenate([res.results[c]["y"] for c in range(NCORES)], axis=0)
    y = y.reshape(T, B, O)
    S1 = sum(float(res.results[c]["lparts"][0, 0]) for c in range(NCORES))
    S2 = sum(float(res.results[c]["lparts"][0, 1]) for c in range(NCORES))
    loss = np.float32(
        COMMIT * (0.5 * TAU * S1
                  + T * B * (RD / 2.0) * math.log(2.0 * math.pi * TAU)
                  - S2)
    )
    return (y, loss), res


def kernel(**inputs):
    outputs, _ = _run(inputs, trace=False)
    return outputs


# revision 7
# speedup vs baseline: 1.4408x; 1.0801x over previous
"""Trainium2 Bass kernel for nn_ConditionalDLFactorized16 (moe_routing).

Data-parallel over tokens: the T*B=16384 tokens are split into 8 contiguous
slabs of 2048, one per NeuronCore. Weights (pw_w/centroids/map_w) are
replicated. The only cross-core quantity is the scalar loss, combined on host
from per-core partial sums.

Per-core math (tokens n local):
  k[n,r]   = x[n,:] @ map_w.T + map_b                    (fp32r matmuls)
  g[n,e]   = k[n,:] @ centroids.T - 0.5*||c_e||^2 + log prior_e
             (bias row folded into an augmented [65,8] centroid matrix,
              with k augmented by a constant-1 row)
  resp     = softmax_e(g)     (the -0.5*kk and const terms cancel in softmax)
  y[n,:]   = sum_e resp[n,e] * (x[n,:] @ W'_e.T) + bias_b
             where W'_e = pw_w[e] + bias_w  (valid because sum_e resp = 1)
  loss parts: S1 = sum_n ||k_n||^2,  S2 = sum_n log(sum_e exp(g[n,e]))
  loss     = 0.1 * (0.5*S1 + N*(RD/2)*log(2*pi) - S2)   (TAU=1)

Perf notes:
  - Expert matmuls in fp32r: full PE rate at free-dim 512, ~2e-4 rel err.
  - Tokens sit on PSUM partitions so the whole resp-mix per expert is ONE
    DVE scalar_tensor_tensor: acc = (z_psum * resp_col) + acc.
  - Inputs stream in small chunks (weights per expert) so PE starts ~4us in
    instead of waiting ~35us for the full 12.4MB load.
  - ACT does only 16 Exp (+accumulated sumexp) and one [128,16] Ln at the
    end: activation-table loads stay at 2 instead of 28.
"""

import math
import sys

import numpy as np

for _p in ("/opt/trn_rl_repo", "/opt/pypackages"):
    if _p not in sys.path:
        sys.path.append(_p)

import concourse.bacc as bacc
import concourse.mybir as mybir
import concourse.tile as tile
from concourse.bass_utils import run_bass_kernel_spmd

F32 = mybir.dt.float32
F32R = mybir.dt.float32r
AF = mybir.ActivationFunctionType
ALU = mybir.AluOpType

T, B, C, O, RD, NE = 2048, 8, 512, 512, 64, 8
TAU, COMMIT = 1.0, 0.1
NCORES = 8
NLOC = T * B // NCORES          # 2048 tokens per core
NT = NLOC // 128                # 16 token tiles of 128
NST = NLOC // 512               # 4 streaming tiles of 512
KC = C // 128                   # 4 contraction chunks

_nc_cache = None


def _build():
    nc = bacc.Bacc(trn_type="TRN2", target_bir_lowering=False)

    xt = nc.dram_tensor("xt", [128, NST, KC, 512], F32R, kind="ExternalInput")
    wt = nc.dram_tensor("wt", [128, NE, KC, O], F32R, kind="ExternalInput")
    mapw = nc.dram_tensor("mapw", [128, KC, RD], F32R, kind="ExternalInput")
    mapb = nc.dram_tensor("mapb", [RD, 1], F32, kind="ExternalInput")
    cent = nc.dram_tensor("cent", [RD + 1, NE], F32, kind="ExternalInput")
    biasb = nc.dram_tensor("biasb", [O], F32, kind="ExternalInput")

    y = nc.dram_tensor("y", [NLOC, O], F32, kind="ExternalOutput")
    lparts = nc.dram_tensor("lparts", [1, 2], F32, kind="ExternalOutput")

    with tile.TileContext(nc) as tc:
        with tc.tile_pool(name="big", bufs=1) as big, \
             tc.tile_pool(name="small", bufs=1) as small, \
             tc.tile_pool(name="acc_p", bufs=1) as acc_p, \
             tc.tile_pool(name="route_p", bufs=16) as route_p, \
             tc.tile_pool(name="ps_z", bufs=6, space="PSUM") as ps_z, \
             tc.tile_pool(name="ps_s", bufs=1, space="PSUM") as ps_s:

            # ---- resident inputs; small tensors first, then streamed chunks
            mapw_sb = small.tile([128, KC, RD], F32R)
            mapb_sb = small.tile([RD, 1], F32)
            cent_sb = small.tile([RD + 1, NE], F32)
            biasbc = small.tile([128, O], F32)
            nc.sync.dma_start(mapw_sb[:], mapw[:])
            nc.sync.dma_start(mapb_sb[:], mapb[:])
            nc.sync.dma_start(cent_sb[:], cent[:])
            nc.sync.dma_start(biasbc[:], biasb[:].partition_broadcast(128))

            xt_sb = big.tile([128, NST, KC, 512], F32R)
            wt_sb = big.tile([128, NE, KC, O], F32R)
            nc.sync.dma_start(xt_sb[:, 0], xt[:, 0])
            nc.sync.dma_start(wt_sb[:, 0], wt[:, 0])
            for st in range(1, NST):
                nc.sync.dma_start(xt_sb[:, st], xt[:, st])
            for e in range(1, NE):
                nc.sync.dma_start(wt_sb[:, e], wt[:, e])

            ones64 = small.tile([RD, 1], F32)
            ones128 = small.tile([128, 1], F32)
            nc.vector.memset(ones64[:], 1.0)
            nc.vector.memset(ones128[:], 1.0)

            # k^T in [r, tok] layout, augmented with a constant-1 row 64
            ktsb = small.tile([RD + 1, NLOC], F32)
            nc.vector.memset(ktsb[RD : RD + 1, :], 1.0)
            sq_cols = small.tile([RD, NST], F32)
            sumexp_cols = small.tile([128, NT], F32)

            # ---- routing: k, then g/expg/resp per 128-token tile, interleaved
            # with xt chunk arrival (kt(st) only needs xt[:, st])
            resp_tiles = [None] * NT
            for st in range(NST):
                kt_ps = ps_s.tile([RD, 512], F32, tag="kt")
                for kc in range(KC):
                    nc.tensor.matmul(
                        kt_ps[:],
                        mapw_sb[:, kc, :],
                        xt_sb[:, st, kc, :],
                        start=(kc == 0),
                        stop=(kc == KC - 1),
                    )
                ksl = ktsb[:RD, st * 512 : (st + 1) * 512]
                nc.vector.tensor_scalar_add(ksl, kt_ps[:], mapb_sb[:])
                # sum_tok k^2 per r-partition (loss S1), square fused with sum
                ksq = route_p.tile([RD, 512], F32, tag="ksq")
                nc.vector.scalar_tensor_tensor(
                    ksq[:], ksl, 1.0, ksl, op0=ALU.mult, op1=ALU.mult,
                    accum_out=sq_cols[:, st : st + 1],
                )
                for t in range(st * NT // NST, (st + 1) * NT // NST):
                    g_ps = ps_s.tile([128, NE], F32, tag="g")
                    nc.tensor.matmul(
                        g_ps[:],
                        ktsb[:, t * 128 : (t + 1) * 128],
                        cent_sb[:],
                        start=True,
                        stop=True,
                    )
                    expg = route_p.tile([128, NE], F32, tag="expg")
                    nc.scalar.activation(
                        expg[:], g_ps[:], AF.Exp, bias=0.0, scale=1.0,
                        accum_out=sumexp_cols[:, t : t + 1],
                    )
                    inv = route_p.tile([128, 1], F32, tag="inv")
                    nc.vector.reciprocal(inv[:], sumexp_cols[:, t : t + 1])
                    resp = route_p.tile([128, NE], F32, tag="resp")
                    nc.vector.tensor_scalar_mul(resp[:], expg[:], inv[:])
                    resp_tiles[t] = resp

            # ---- loss partial sums (emitted early so nothing trails the mix)
            lse16 = small.tile([128, NT], F32)
            nc.scalar.activation(lse16[:], sumexp_cols[:], AF.Ln, bias=0.0, scale=1.0)
            sq_red = small.tile([RD, 1], F32)
            lse_red = small.tile([128, 1], F32)
            nc.vector.reduce_sum(sq_red[:], sq_cols[:], axis=mybir.AxisListType.X)
            nc.vector.reduce_sum(lse_red[:], lse16[:], axis=mybir.AxisListType.X)
            l1_ps = ps_s.tile([1, 1], F32, tag="kt")
            l2_ps = ps_s.tile([1, 1], F32, tag="g")
            nc.tensor.matmul(l1_ps[:], sq_red[:], ones64[:], start=True, stop=True)
            nc.tensor.matmul(l2_ps[:], lse_red[:], ones128[:], start=True, stop=True)
            lp_sb = small.tile([1, 2], F32)
            nc.vector.tensor_copy(lp_sb[:, 0:1], l1_ps[:])
            nc.vector.tensor_copy(lp_sb[:, 1:2], l2_ps[:])
            nc.sync.dma_start(lparts[:], lp_sb[:])

            # ---- expert matmuls + resp mixing, expert-OUTER so wt[e] DMA
            # arrival never stalls the PE; one DVE op per (expert, tile)
            accs = [acc_p.tile([128, O], F32, tag=f"acc{t}", name=f"acc{t}") for t in range(NT)]
            for e in range(NE):
                for t in range(NT):
                    st, m0 = t // (NT // NST), (t % (NT // NST)) * 128
                    z_ps = ps_z.tile([128, O], F32, tag="z")
                    for kc in range(KC):
                        nc.tensor.matmul(
                            z_ps[:],
                            xt_sb[:, st, kc, m0 : m0 + 128],
                            wt_sb[:, e, kc, :],
                            start=(kc == 0),
                            stop=(kc == KC - 1),
                        )
                    nc.vector.scalar_tensor_tensor(
                        accs[t][:], z_ps[:], resp_tiles[t][:, e : e + 1],
                        biasbc[:] if e == 0 else accs[t][:],
                        op0=ALU.mult, op1=ALU.add,
                    )
                    if e == NE - 1:
                        nc.sync.dma_start(y[t * 128 : (t + 1) * 128, :], accs[t][:])

    nc.finalize()
    return nc


def _get_nc():
    global _nc_cache
    if _nc_cache is None:
        _nc_cache = _build()
    return _nc_cache


def _prep_inputs(x, key_feat, map_w, map_b, centroids, prior, pw_w, bias_w, bias_b):
    """Host-side layout: shard tokens, transpose into c-on-partition chunks."""
    xf = np.ascontiguousarray(x, dtype=np.float32).reshape(T * B, C)

    Wp = pw_w.reshape(NE, O, C).astype(np.float32) + bias_w.astype(np.float32)[None]
    # wt[ci, e, kc, o] = Wp[e, o, kc*128+ci]
    wt = np.ascontiguousarray(Wp.reshape(NE, O, KC, 128).transpose(3, 0, 2, 1))
    # mapw[ci, kc, r] = map_w[r, kc*128+ci]
    mapw = np.ascontiguousarray(
        map_w.astype(np.float32).reshape(RD, KC, 128).transpose(2, 1, 0)
    )
    mapb = np.ascontiguousarray(map_b.astype(np.float32).reshape(RD, 1))
    cent = np.concatenate(
        [
            centroids.astype(np.float32).T,
            (-0.5 * (centroids.astype(np.float32) ** 2).sum(1)
             + np.log(prior[0].astype(np.float32)))[None, :],
        ],
        axis=0,
    ).astype(np.float32)
    biasb = np.ascontiguousarray(bias_b, dtype=np.float32)

    in_maps = []
    for c in range(NCORES):
        xc = xf[c * NLOC : (c + 1) * NLOC]
        # xt[ci, st, kc, m] = xc[st*512 + m, kc*128+ci]
        xtc = np.ascontiguousarray(
            xc.reshape(NST, 512, KC, 128).transpose(3, 0, 2, 1)
        )
        in_maps.append(
            dict(xt=xtc, wt=wt, mapw=mapw, mapb=mapb, cent=cent, biasb=biasb)
        )
    return in_maps


def _run(inputs, trace=False, tmpdir=None):
    in_maps = _prep_inputs(**inputs)
    nc = _get_nc()
    res = run_bass_kernel_spmd(
        nc, in_maps, core_ids=list(range(NCORES)), trace=trace, tmpdir=tmpdir
    )
    y = np.concatenate([res.results[c]["y"] for c in range(NCORES)], axis=0)
    y = y.reshape(T, B, O)
    S1 = sum(float(res.results[c]["lparts"][0, 0]) for c in range(NCORES))
    S2 = sum(float(res.results[c]["lparts"][0, 1]) for c in range(NCORES))
    loss = np.float32(
        COMMIT * (0.5 * TAU * S1
                  + T * B * (RD / 2.0) * math.log(2.0 * math.pi * TAU)
                  - S2)
    )
    return (y, loss), res


def kernel(**inputs):
    outputs, _ = _run(inputs, trace=False)
    return outputs


# revision 8
# speedup vs baseline: 1.4744x; 1.0233x over previous
"""Trainium2 Bass kernel for nn_ConditionalDLFactorized16 (moe_routing).

Data-parallel over tokens: the T*B=16384 tokens are split into 8 contiguous
slabs of 2048, one per NeuronCore. Weights (pw_w/centroids/map_w) are
replicated. The only cross-core quantity is the scalar loss, combined on host
from per-core partial sums.

Per-core math (tokens n local):
  k[n,r]   = x[n,:] @ map_w.T + map_b                    (fp32r matmuls)
  g[n,e]   = k[n,:] @ centroids.T - 0.5*||c_e||^2 + log prior_e
             (bias row folded into an augmented [65,8] centroid matrix,
              with k augmented by a constant-1 row)
  resp     = softmax_e(g)     (the -0.5*kk and const terms cancel in softmax)
  y[n,:]   = sum_e resp[n,e] * (x[n,:] @ W'_e.T) + bias_b
             where W'_e = pw_w[e] + bias_w  (valid because sum_e resp = 1)
  loss parts: S1 = sum_n ||k_n||^2,  S2 = sum_n log(sum_e exp(g[n,e]))
  loss     = 0.1 * (0.5*S1 + N*(RD/2)*log(2*pi) - S2)   (TAU=1)

Perf notes:
  - Expert matmuls in fp32r: full PE rate at free-dim 512, ~2e-4 rel err.
  - Tokens sit on PSUM partitions so the whole resp-mix per expert is ONE
    DVE scalar_tensor_tensor: acc = (z_psum * resp_col) + acc.
  - Inputs stream in small chunks (weights per expert) so PE starts ~4us in
    instead of waiting ~35us for the full 12.4MB load.
  - ACT does only 16 Exp (+accumulated sumexp) and one [128,16] Ln at the
    end: activation-table loads stay at 2 instead of 28.
"""

import math
import sys

import numpy as np

for _p in ("/opt/trn_rl_repo", "/opt/pypackages"):
    if _p not in sys.path:
        sys.path.append(_p)

import concourse.bacc as bacc
import concourse.mybir as mybir
import concourse.tile as tile
from concourse.bass_utils import run_bass_kernel_spmd

F32 = mybir.dt.float32
F32R = mybir.dt.float32r
AF = mybir.ActivationFunctionType
ALU = mybir.AluOpType

T, B, C, O, RD, NE = 2048, 8, 512, 512, 64, 8
TAU, COMMIT = 1.0, 0.1
NCORES = 8
NLOC = T * B // NCORES          # 2048 tokens per core
NT = NLOC // 128                # 16 token tiles of 128
NST = NLOC // 512               # 4 streaming tiles of 512
KC = C // 128                   # 4 contraction chunks

_nc_cache = None


def _build():
    nc = bacc.Bacc(trn_type="TRN2", target_bir_lowering=False)

    xt = nc.dram_tensor("xt", [128, NST, KC, 512], F32R, kind="ExternalInput")
    wt = nc.dram_tensor("wt", [128, NE, KC, O], F32R, kind="ExternalInput")
    mapw = nc.dram_tensor("mapw", [128, KC, RD], F32R, kind="ExternalInput")
    mapb = nc.dram_tensor("mapb", [RD, 1], F32, kind="ExternalInput")
    cent = nc.dram_tensor("cent", [RD + 1, NE], F32, kind="ExternalInput")
    biasb = nc.dram_tensor("biasb", [O], F32, kind="ExternalInput")

    y = nc.dram_tensor("y", [NLOC, O], F32, kind="ExternalOutput")
    lparts = nc.dram_tensor("lparts", [1, 2], F32, kind="ExternalOutput")

    with tile.TileContext(nc) as tc:
        with tc.tile_pool(name="big", bufs=1) as big, \
             tc.tile_pool(name="small", bufs=1) as small, \
             tc.tile_pool(name="acc_p", bufs=1) as acc_p, \
             tc.tile_pool(name="route_p", bufs=16) as route_p, \
             tc.tile_pool(name="ps_z", bufs=6, space="PSUM") as ps_z, \
             tc.tile_pool(name="ps_s", bufs=1, space="PSUM") as ps_s:

            # ---- resident inputs; small tensors first, then streamed chunks
            mapw_sb = small.tile([128, KC, RD], F32R)
            mapb_sb = small.tile([RD, 1], F32)
            cent_sb = small.tile([RD + 1, NE], F32)
            biasbc = small.tile([128, O], F32)
            xt_sb = big.tile([128, NST, KC, 512], F32R)
            wt_sb = big.tile([128, NE, KC, O], F32R)
            # issue order = need order: kt0 needs only mapw + xt0
            nc.sync.dma_start(mapw_sb[:], mapw[:])
            nc.sync.dma_start(xt_sb[:, 0], xt[:, 0])
            nc.sync.dma_start(mapb_sb[:], mapb[:])
            nc.sync.dma_start(cent_sb[:], cent[:])
            nc.sync.dma_start(wt_sb[:, 0], wt[:, 0])
            nc.sync.dma_start(biasbc[:], biasb[:].partition_broadcast(128))
            for st in range(1, NST):
                nc.sync.dma_start(xt_sb[:, st], xt[:, st])
            for e in range(1, NE):
                nc.sync.dma_start(wt_sb[:, e], wt[:, e])

            ones64 = small.tile([RD, 1], F32)
            ones128 = small.tile([128, 1], F32)
            nc.vector.memset(ones64[:], 1.0)
            nc.vector.memset(ones128[:], 1.0)

            # k^T in [r, tok] layout, augmented with a constant-1 row 64
            ktsb = small.tile([RD + 1, NLOC], F32)
            nc.vector.memset(ktsb[RD : RD + 1, :], 1.0)
            sq_cols = small.tile([RD, NST], F32)
            sumexp_cols = small.tile([128, NT], F32)

            # ---- routing: k, then g/expg/resp per 128-token tile, interleaved
            # with xt chunk arrival (kt(st) only needs xt[:, st])
            resp_tiles = [None] * NT
            for st in range(NST):
                kt_ps = ps_s.tile([RD, 512], F32, tag="kt")
                for kc in range(KC):
                    nc.tensor.matmul(
                        kt_ps[:],
                        mapw_sb[:, kc, :],
                        xt_sb[:, st, kc, :],
                        start=(kc == 0),
                        stop=(kc == KC - 1),
                    )
                ksl = ktsb[:RD, st * 512 : (st + 1) * 512]
                nc.vector.tensor_scalar_add(ksl, kt_ps[:], mapb_sb[:])
                # sum_tok k^2 per r-partition (loss S1), square fused with sum
                ksq = route_p.tile([RD, 512], F32, tag="ksq")
                nc.vector.scalar_tensor_tensor(
                    ksq[:], ksl, 1.0, ksl, op0=ALU.mult, op1=ALU.mult,
                    accum_out=sq_cols[:, st : st + 1],
                )
                for t in range(st * NT // NST, (st + 1) * NT // NST):
                    g_ps = ps_s.tile([128, NE], F32, tag="g")
                    nc.tensor.matmul(
                        g_ps[:],
                        ktsb[:, t * 128 : (t + 1) * 128],
                        cent_sb[:],
                        start=True,
                        stop=True,
                    )
                    expg = route_p.tile([128, NE], F32, tag="expg")
                    nc.scalar.activation(
                        expg[:], g_ps[:], AF.Exp, bias=0.0, scale=1.0,
                        accum_out=sumexp_cols[:, t : t + 1],
                    )
                    inv = route_p.tile([128, 1], F32, tag="inv")
                    nc.vector.reciprocal(inv[:], sumexp_cols[:, t : t + 1])
                    resp = route_p.tile([128, NE], F32, tag="resp")
                    nc.vector.tensor_scalar_mul(resp[:], expg[:], inv[:])
                    resp_tiles[t] = resp

            # ---- loss partial sums (emitted early so nothing trails the mix)
            lse16 = small.tile([128, NT], F32)
            nc.scalar.activation(lse16[:], sumexp_cols[:], AF.Ln, bias=0.0, scale=1.0)
            sq_red = small.tile([RD, 1], F32)
            lse_red = small.tile([128, 1], F32)
            nc.vector.reduce_sum(sq_red[:], sq_cols[:], axis=mybir.AxisListType.X)
            nc.vector.reduce_sum(lse_red[:], lse16[:], axis=mybir.AxisListType.X)
            l1_ps = ps_s.tile([1, 1], F32, tag="kt")
            l2_ps = ps_s.tile([1, 1], F32, tag="g")
            nc.tensor.matmul(l1_ps[:], sq_red[:], ones64[:], start=True, stop=True)
            nc.tensor.matmul(l2_ps[:], lse_red[:], ones128[:], start=True, stop=True)
            lp_sb = small.tile([1, 2], F32)
            nc.vector.tensor_copy(lp_sb[:, 0:1], l1_ps[:])
            nc.vector.tensor_copy(lp_sb[:, 1:2], l2_ps[:])
            nc.sync.dma_start(lparts[:], lp_sb[:])

            # ---- expert matmuls + resp mixing, expert-OUTER so wt[e] DMA
            # arrival never stalls the PE; one DVE op per (expert, tile)
            accs = [acc_p.tile([128, O], F32, tag=f"acc{t}", name=f"acc{t}") for t in range(NT)]
            for e in range(NE):
                for t in range(NT):
                    st, m0 = t // (NT // NST), (t % (NT // NST)) * 128
                    z_ps = ps_z.tile([128, O], F32, tag="z")
                    for kc in range(KC):
                        nc.tensor.matmul(
                            z_ps[:],
                            xt_sb[:, st, kc, m0 : m0 + 128],
                            wt_sb[:, e, kc, :],
                            start=(kc == 0),
                            stop=(kc == KC - 1),
                        )
                    nc.vector.scalar_tensor_tensor(
                        accs[t][:], z_ps[:], resp_tiles[t][:, e : e + 1],
                        biasbc[:] if e == 0 else accs[t][:],
                        op0=ALU.mult, op1=ALU.add,
                    )
                    if e == NE - 1:
                        nc.sync.dma_start(y[t * 128 : (t + 1) * 128, :], accs[t][:])

    nc.finalize()
    return nc


def _get_nc():
    global _nc_cache
    if _nc_cache is None:
        _nc_cache = _build()
    return _nc_cache


def _prep_inputs(x, key_feat, map_w, map_b, centroids, prior, pw_w, bias_w, bias_b):
    """Host-side layout: shard tokens, transpose into c-on-partition chunks."""
    xf = np.ascontiguousarray(x, dtype=np.float32).reshape(T * B, C)

    Wp = pw_w.reshape(NE, O, C).astype(np.float32) + bias_w.astype(np.float32)[None]
    # wt[ci, e, kc, o] = Wp[e, o, kc*128+ci]
    wt = np.ascontiguousarray(Wp.reshape(NE, O, KC, 128).transpose(3, 0, 2, 1))
    # mapw[ci, kc, r] = map_w[r, kc*128+ci]
    mapw = np.ascontiguousarray(
        map_w.astype(np.float32).reshape(RD, KC, 128).transpose(2, 1, 0)
    )
    mapb = np.ascontiguousarray(map_b.astype(np.float32).reshape(RD, 1))
    cent = np.concatenate(
        [
            centroids.astype(np.float32).T,
            (-0.5 * (centroids.astype(np.float32) ** 2).sum(1)
             + np.log(prior[0].astype(np.float32)))[None, :],
        ],
        axis=0,
    ).astype(np.float32)
    biasb = np.ascontiguousarray(bias_b, dtype=np.float32)

    in_maps = []
    for c in range(NCORES):
        xc = xf[c * NLOC : (c + 1) * NLOC]
        # xt[ci, st, kc, m] = xc[st*512 + m, kc*128+ci]
        xtc = np.ascontiguousarray(
            xc.reshape(NST, 512, KC, 128).transpose(3, 0, 2, 1)
        )
        in_maps.append(
            dict(xt=xtc, wt=wt, mapw=mapw, mapb=mapb, cent=cent, biasb=biasb)
        )
    return in_maps


def _run(inputs, trace=False, tmpdir=None):
    in_maps = _prep_inputs(**inputs)
    nc = _get_nc()
    res = run_bass_kernel_spmd(
        nc, in_maps, core_ids=list(range(NCORES)), trace=trace, tmpdir=tmpdir
    )
    y = np.concatenate([res.results[c]["y"] for c in range(NCORES)], axis=0)
    y = y.reshape(T, B, O)
    S1 = sum(float(res.results[c]["lparts"][0, 0]) for c in range(NCORES))
    S2 = sum(float(res.results[c]["lparts"][0, 1]) for c in range(NCORES))
    loss = np.float32(
        COMMIT * (0.5 * TAU * S1
                  + T * B * (RD / 2.0) * math.log(2.0 * math.pi * TAU)
                  - S2)
    )
    return (y, loss), res


def kernel(**inputs):
    outputs, _ = _run(inputs, trace=False)
    return outputs


# revision 9
# speedup vs baseline: 1.4882x; 1.0094x over previous
"""Trainium2 Bass kernel for nn_ConditionalDLFactorized16 (moe_routing).

Data-parallel over tokens: the T*B=16384 tokens are split into 8 contiguous
slabs of 2048, one per NeuronCore. Weights (pw_w/centroids/map_w) are
replicated. The only cross-core quantity is the scalar loss, combined on host
from per-core partial sums.

Per-core math (tokens n local):
  k[n,r]   = x[n,:] @ map_w.T + map_b                    (fp32r matmuls)
  g[n,e]   = k[n,:] @ centroids.T - 0.5*||c_e||^2 + log prior_e
             (bias row folded into an augmented [65,8] centroid matrix,
              with k augmented by a constant-1 row)
  resp     = softmax_e(g)     (the -0.5*kk and const terms cancel in softmax)
  y[n,:]   = sum_e resp[n,e] * (x[n,:] @ W'_e.T) + bias_b
             where W'_e = pw_w[e] + bias_w  (valid because sum_e resp = 1)
  loss parts: S1 = sum_n ||k_n||^2,  S2 = sum_n log(sum_e exp(g[n,e]))
  loss     = 0.1 * (0.5*S1 + N*(RD/2)*log(2*pi) - S2)   (TAU=1)

Perf notes:
  - Expert matmuls in fp32r: full PE rate at free-dim 512, ~2e-4 rel err.
  - Tokens sit on PSUM partitions so the whole resp-mix per expert is ONE
    DVE scalar_tensor_tensor: acc = (z_psum * resp_col) + acc.
  - Inputs stream in small chunks (weights per expert) so PE starts ~4us in
    instead of waiting ~35us for the full 12.4MB load.
  - ACT does only 16 Exp (+accumulated sumexp) and one [128,16] Ln at the
    end: activation-table loads stay at 2 instead of 28.
"""

import math
import sys

import numpy as np

for _p in ("/opt/trn_rl_repo", "/opt/pypackages"):
    if _p not in sys.path:
        sys.path.append(_p)

import concourse.bacc as bacc
import concourse.mybir as mybir
import concourse.tile as tile
from concourse.bass_utils import run_bass_kernel_spmd

F32 = mybir.dt.float32
F32R = mybir.dt.float32r
AF = mybir.ActivationFunctionType
ALU = mybir.AluOpType

T, B, C, O, RD, NE = 2048, 8, 512, 512, 64, 8
TAU, COMMIT = 1.0, 0.1
NCORES = 8
NLOC = T * B // NCORES          # 2048 tokens per core
NT = NLOC // 128                # 16 token tiles of 128
NST = NLOC // 512               # 4 streaming tiles of 512
KC = C // 128                   # 4 contraction chunks

_nc_cache = None


def _build():
    nc = bacc.Bacc(trn_type="TRN2", target_bir_lowering=False)

    xt = nc.dram_tensor("xt", [128, NST, KC, 512], F32R, kind="ExternalInput")
    wt = nc.dram_tensor("wt", [128, NE, KC, O], F32R, kind="ExternalInput")
    mapw = nc.dram_tensor("mapw", [128, KC, RD], F32R, kind="ExternalInput")
    mapb = nc.dram_tensor("mapb", [RD, 1], F32, kind="ExternalInput")
    cent = nc.dram_tensor("cent", [RD + 1, NE], F32, kind="ExternalInput")
    biasb = nc.dram_tensor("biasb", [O], F32, kind="ExternalInput")

    y = nc.dram_tensor("y", [NLOC, O], F32, kind="ExternalOutput")
    lparts = nc.dram_tensor("lparts", [1, 2], F32, kind="ExternalOutput")

    with tile.TileContext(nc) as tc:
        with tc.tile_pool(name="big", bufs=1) as big, \
             tc.tile_pool(name="small", bufs=1) as small, \
             tc.tile_pool(name="acc_p", bufs=1) as acc_p, \
             tc.tile_pool(name="route_p", bufs=16) as route_p, \
             tc.tile_pool(name="ps_z", bufs=6, space="PSUM") as ps_z, \
             tc.tile_pool(name="ps_s", bufs=1, space="PSUM") as ps_s:

            # ---- resident inputs; small tensors first, then streamed chunks
            mapw_sb = small.tile([128, KC, RD], F32R)
            mapb_sb = small.tile([RD, 1], F32)
            cent_sb = small.tile([RD + 1, NE], F32)
            biasbc = small.tile([128, O], F32)
            xt_sb = big.tile([128, NST, KC, 512], F32R)
            wt_sb = big.tile([128, NE, KC, O], F32R)
            # issue order = need order: kt0 needs only mapw + xt0; the first
            # xt/wt chunks are further split by kc so matmuls start sooner
            nc.sync.dma_start(mapw_sb[:], mapw[:])
            for kc in range(KC):
                nc.sync.dma_start(xt_sb[:, 0, kc], xt[:, 0, kc])
            nc.sync.dma_start(mapb_sb[:], mapb[:])
            nc.sync.dma_start(cent_sb[:], cent[:])
            for kc in range(KC):
                nc.sync.dma_start(wt_sb[:, 0, kc], wt[:, 0, kc])
            nc.sync.dma_start(biasbc[:], biasb[:].partition_broadcast(128))
            for st in range(1, NST):
                nc.sync.dma_start(xt_sb[:, st], xt[:, st])
            for e in range(1, NE):
                nc.sync.dma_start(wt_sb[:, e], wt[:, e])

            ones64 = small.tile([RD, 1], F32)
            ones128 = small.tile([128, 1], F32)
            nc.vector.memset(ones64[:], 1.0)
            nc.vector.memset(ones128[:], 1.0)

            # k^T in [r, tok] layout, augmented with a constant-1 row 64
            ktsb = small.tile([RD + 1, NLOC], F32)
            nc.vector.memset(ktsb[RD : RD + 1, :], 1.0)
            sq_cols = small.tile([RD, NST], F32)
            sumexp_cols = small.tile([128, NT], F32)

            # ---- routing: k, then g/expg/resp per 128-token tile, interleaved
            # with xt chunk arrival (kt(st) only needs xt[:, st])
            resp_tiles = [None] * NT
            for st in range(NST):
                kt_ps = ps_s.tile([RD, 512], F32, tag="kt")
                for kc in range(KC):
                    nc.tensor.matmul(
                        kt_ps[:],
                        mapw_sb[:, kc, :],
                        xt_sb[:, st, kc, :],
                        start=(kc == 0),
                        stop=(kc == KC - 1),
                    )
                ksl = ktsb[:RD, st * 512 : (st + 1) * 512]
                nc.vector.tensor_scalar_add(ksl, kt_ps[:], mapb_sb[:])
                # sum_tok k^2 per r-partition (loss S1), square fused with sum
                ksq = route_p.tile([RD, 512], F32, tag="ksq")
                nc.vector.scalar_tensor_tensor(
                    ksq[:], ksl, 1.0, ksl, op0=ALU.mult, op1=ALU.mult,
                    accum_out=sq_cols[:, st : st + 1],
                )
                for t in range(st * NT // NST, (st + 1) * NT // NST):
                    g_ps = ps_s.tile([128, NE], F32, tag="g")
                    nc.tensor.matmul(
                        g_ps[:],
                        ktsb[:, t * 128 : (t + 1) * 128],
                        cent_sb[:],
                        start=True,
                        stop=True,
                    )
                    expg = route_p.tile([128, NE], F32, tag="expg")
                    nc.scalar.activation(
                        expg[:], g_ps[:], AF.Exp, bias=0.0, scale=1.0,
                        accum_out=sumexp_cols[:, t : t + 1],
                    )
                    inv = route_p.tile([128, 1], F32, tag="inv")
                    nc.vector.reciprocal(inv[:], sumexp_cols[:, t : t + 1])
                    resp = route_p.tile([128, NE], F32, tag="resp")
                    nc.vector.tensor_scalar_mul(resp[:], expg[:], inv[:])
                    resp_tiles[t] = resp

            # ---- loss partial sums (emitted early so nothing trails the mix)
            lse16 = small.tile([128, NT], F32)
            nc.scalar.activation(lse16[:], sumexp_cols[:], AF.Ln, bias=0.0, scale=1.0)
            sq_red = small.tile([RD, 1], F32)
            lse_red = small.tile([128, 1], F32)
            nc.vector.reduce_sum(sq_red[:], sq_cols[:], axis=mybir.AxisListType.X)
            nc.vector.reduce_sum(lse_red[:], lse16[:], axis=mybir.AxisListType.X)
            l1_ps = ps_s.tile([1, 1], F32, tag="kt")
            l2_ps = ps_s.tile([1, 1], F32, tag="g")
            nc.tensor.matmul(l1_ps[:], sq_red[:], ones64[:], start=True, stop=True)
            nc.tensor.matmul(l2_ps[:], lse_red[:], ones128[:], start=True, stop=True)
            lp_sb = small.tile([1, 2], F32)
            nc.vector.tensor_copy(lp_sb[:, 0:1], l1_ps[:])
            nc.vector.tensor_copy(lp_sb[:, 1:2], l2_ps[:])
            nc.sync.dma_start(lparts[:], lp_sb[:])

            # ---- expert matmuls + resp mixing, expert-OUTER so wt[e] DMA
            # arrival never stalls the PE; one DVE op per (expert, tile)
            accs = [acc_p.tile([128, O], F32, tag=f"acc{t}", name=f"acc{t}") for t in range(NT)]
            for e in range(NE):
                for t in range(NT):
                    st, m0 = t // (NT // NST), (t % (NT // NST)) * 128
                    z_ps = ps_z.tile([128, O], F32, tag="z")
                    for kc in range(KC):
                        nc.tensor.matmul(
                            z_ps[:],
                            xt_sb[:, st, kc, m0 : m0 + 128],
                            wt_sb[:, e, kc, :],
                            start=(kc == 0),
                            stop=(kc == KC - 1),
                        )
                    nc.vector.scalar_tensor_tensor(
                        accs[t][:], z_ps[:], resp_tiles[t][:, e : e + 1],
                        biasbc[:] if e == 0 else accs[t][:],
                        op0=ALU.mult, op1=ALU.add,
                    )
                    if e == NE - 1:
                        nc.sync.dma_start(y[t * 128 : (t + 1) * 128, :], accs[t][:])

    nc.finalize()
    return nc


def _get_nc():
    global _nc_cache
    if _nc_cache is None:
        _nc_cache = _build()
    return _nc_cache


def _prep_inputs(x, key_feat, map_w, map_b, centroids, prior, pw_w, bias_w, bias_b):
    """Host-side layout: shard tokens, transpose into c-on-partition chunks."""
    xf = np.ascontiguousarray(x, dtype=np.float32).reshape(T * B, C)

    Wp = pw_w.reshape(NE, O, C).astype(np.float32) + bias_w.astype(np.float32)[None]
    # wt[ci, e, kc, o] = Wp[e, o, kc*128+ci]
    wt = np.ascontiguousarray(Wp.reshape(NE, O, KC, 128).transpose(3, 0, 2, 1))
    # mapw[ci, kc, r] = map_w[r, kc*128+ci]
    mapw = np.ascontiguousarray(
        map_w.astype(np.float32).reshape(RD, KC, 128).transpose(2, 1, 0)
    )
    mapb = np.ascontiguousarray(map_b.astype(np.float32).reshape(RD, 1))
    cent = np.concatenate(
        [
            centroids.astype(np.float32).T,
            (-0.5 * (centroids.astype(np.float32) ** 2).sum(1)
             + np.log(prior[0].astype(np.float32)))[None, :],
        ],
        axis=0,
    ).astype(np.float32)
    biasb = np.ascontiguousarray(bias_b, dtype=np.float32)

    in_maps = []
    for c in range(NCORES):
        xc = xf[c * NLOC : (c + 1) * NLOC]
        # xt[ci, st, kc, m] = xc[st*512 + m, kc*128+ci]
        xtc = np.ascontiguousarray(
            xc.reshape(NST, 512, KC, 128).transpose(3, 0, 2, 1)
        )
        in_maps.append(
            dict(xt=xtc, wt=wt, mapw=mapw, mapb=mapb, cent=cent, biasb=biasb)
        )
    return in_maps


def _run(inputs, trace=False, tmpdir=None):
    in_maps = _prep_inputs(**inputs)
    nc = _get_nc()
    res = run_bass_kernel_spmd(
        nc, in_maps, core_ids=list(range(NCORES)), trace=trace, tmpdir=tmpdir
    )
    y = np.concatenate([res.results[c]["y"] for c in range(NCORES)], axis=0)
    y = y.reshape(T, B, O)
    S1 = sum(float(res.results[c]["lparts"][0, 0]) for c in range(NCORES))
    S2 = sum(float(res.results[c]["lparts"][0, 1]) for c in range(NCORES))
    loss = np.float32(
        COMMIT * (0.5 * TAU * S1
                  + T * B * (RD / 2.0) * math.log(2.0 * math.pi * TAU)
                  - S2)
    )
    return (y, loss), res


def kernel(**inputs):
    outputs, _ = _run(inputs, trace=False)
    return outputs
